# revision 4
# baseline (speedup 1.0000x reference)
"""Trainium2 Bass kernel for nn_KernelDensityLoss (KDE softmax loss).

Math: the reference's O(B^2*D) pairwise log-prob matrix collapses to
per-class sufficient statistics.  For row i and class c,

  sums[i,c] = sum_{n in c} lp[i,n]
            = -0.5*(M*const + (M*sq[i] + Ssq[c] - 2*x_i.S_c)/var)

with S_c = sum of class-c embeddings [D], Ssq[c] = sum of squared norms,
sq[i] = ||x_i||^2.  The -0.5*const shift is identical for the own-class
(leave-one-out) and other-class branches and cancels in
logsumexp(row) - own, so the kernel only computes

  A[i,c] = M*sq[i] + Ssq[c] - 2*G[i,c]        (G = X @ S^T)
  P[i,c] = -0.5*A[i,c] / (var*m_c)            (m_c = M-1 own class, M else)
  loss   = sum_i relu(logsumexp_c P[i,c] - P[i,own])

Distribution: B=7168 rows sharded 896/core across 8 NeuronCores.  Each
core computes partial class stats with PE matmuls against the one-hot
class matrix (lhsT = x_tile -> stats come out directly with D on the
partition axis, no transposes), AllGather + local sum combines them
(lower latency floor than AllReduce), then each core evaluates its own
896 rows and emits a partial loss scalar; the host sums 8 scalars.
"""

import numpy as np

import concourse.bass as bass
import concourse.bacc as bacc
import concourse.mybir as mybir
import concourse.tile as tile
from concourse.bass_utils import run_bass_kernel_spmd

B = 7168      # total rows
C = 7         # classes
M = 1024      # rows per class
D = 256       # embedding dim
NCORES = 8
R = B // NCORES          # 896 rows per core
T = R // 128             # 7 row-tiles of 128 per core

F32 = mybir.dt.float32
AX = mybir.AxisListType
AF = mybir.ActivationFunctionType
ALU = mybir.AluOpType

# stats layout (free dim of the [128, SW] stats tile):
#   cols 0:7    S half0  (class sums for d in [0,128))
#   cols 7:14   S half1  (class sums for d in [128,256))
#   row0 14:21  Ssq row  (per-class sum of squared norms)
SW = 24


def build_program():
    nc = bacc.Bacc(
        "TRN2",
        target_bir_lowering=False,
        debug=False,
        enable_asserts=True,
        num_devices=NCORES,
    )

    x_d = nc.dram_tensor("x", [R, D], F32, kind="ExternalInput")
    xt_d = nc.dram_tensor("xt", [D, R], F32, kind="ExternalInput")
    y_d = nc.dram_tensor("y", [R, C], F32, kind="ExternalInput")
    consts_d = nc.dram_tensor("consts", [128, 4], F32, kind="ExternalInput")
    out_d = nc.dram_tensor("loss_part", [1, 1], F32, kind="ExternalOutput")

    ag_out = nc.dram_tensor("ag_out", [NCORES * 128, SW], F32, addr_space="Shared")

    with tile.TileContext(nc) as tc:
        with (
            tc.tile_pool(name="persist", bufs=1) as pp,
            tc.tile_pool(name="xtiles", bufs=4) as px,
            tc.tile_pool(name="scratch", bufs=2) as ps,
            tc.tile_pool(name="chunk", bufs=2) as pc,
            tc.tile_pool(name="psum_stat", bufs=1, space="PSUM") as qstat,
            tc.tile_pool(name="psum_p", bufs=2, space="PSUM") as qp,
            tc.tile_pool(name="dram", bufs=1, space="DRAM") as pd,
        ):
            # ---- persistent tiles ----
            xt0 = pp.tile([128, R], F32, tag="xt0")      # d in [0,128)
            xt1 = pp.tile([128, R], F32, tag="xt1")      # d in [128,256)
            ytile = pp.tile([128, T, C], F32, tag="y")   # [p, t, c]
            consts = pp.tile([128, 4], F32, tag="consts")
            sq = pp.tile([128, T], F32, tag="sq")        # ||x||^2, col per tile
            b_oth = pp.tile([128, T], F32, tag="b_oth")
            b_own = pp.tile([128, T], F32, tag="b_own")
            stats = pp.tile([128, SW], F32, tag="stats")
            gath = pp.tile([128, NCORES, SW], F32, tag="gath")
            statsR = pp.tile([128, SW], F32, tag="statsR")
            shsc = pp.tile([128, 2 * C], F32, tag="shsc")
            accL = pp.tile([128, T], F32, tag="accL")
            accT = pp.tile([128, 1], F32, tag="accT")
            ones_row = pp.tile([1, 128], F32, tag="ones_row")
            ones_col = pp.tile([128, 1], F32, tag="ones_col")
            out_s = pp.tile([1, 1], F32, tag="out_s")

            ph0 = qstat.tile([128, C], F32, tag="ph0")
            ph1 = qstat.tile([128, C], F32, tag="ph1")
            pssq = qstat.tile([1, C], F32, tag="pssq")
            ploss = qstat.tile([1, 1], F32, tag="ploss")

            cc_in = pd.tile([128, SW], F32, tag="cc_in")

            # ---- loads ----
            nc.sync.dma_start(out=consts[:], in_=consts_d[:, :])
            nc.sync.dma_start(
                out=ytile[:],
                in_=y_d.ap().rearrange("(t p) c -> p t c", p=128),
            )
            for h in range(T):
                lo, hi = h * 128, (h + 1) * 128
                nc.sync.dma_start(out=xt0[:, lo:hi], in_=xt_d[0:128, lo:hi])
                nc.sync.dma_start(out=xt1[:, lo:hi], in_=xt_d[128:256, lo:hi])

            nc.vector.memset(ones_row[:], 1.0)
            nc.vector.memset(ones_col[:], 1.0)
            nc.vector.memset(stats[:], 0.0)

            # ---- phase 1: per-core partial class stats ----
            for t in range(T):
                x_t = px.tile([128, D], F32, tag="x_t")
                nc.sync.dma_start(out=x_t[:], in_=x_d[t * 128:(t + 1) * 128, :])

                # ACT Square + accum_out: sq[:,t] = row-sum of x_t^2
                # (tensor_tensor_reduce faults the exec unit on this runtime)
                xsq = ps.tile([128, D], F32, tag="xsq")
                nc.scalar.activation(xsq[:], x_t[:], AF.Square,
                                     bias=0.0, scale=1.0, accum_out=sq[:, t:t + 1])

                y_t = ytile[:, t, :]
                st = (t == 0)
                sp = (t == T - 1)
                nc.tensor.matmul(ph0[:], lhsT=x_t[:, 0:128], rhs=y_t, start=st, stop=sp)
                nc.tensor.matmul(ph1[:], lhsT=x_t[:, 128:256], rhs=y_t, start=st, stop=sp)
                nc.tensor.matmul(pssq[:], lhsT=sq[:, t:t + 1], rhs=y_t, start=st, stop=sp)

            # biases: b_oth = (-0.5/var)*sq ; b_own = (-0.5*M/(var*(M-1)))*sq
            nc.scalar.activation(b_oth[:], sq[:], AF.Copy, bias=0.0, scale=consts[:, 2:3])
            nc.scalar.activation(b_own[:], sq[:], AF.Copy, bias=0.0, scale=consts[:, 3:4])

            # ---- phase 2: exchange + combine ----
            nc.scalar.copy(stats[:, 0:C], ph0[:])
            nc.scalar.copy(stats[:, C:2 * C], ph1[:])
            nc.scalar.copy(stats[0:1, 2 * C:3 * C], pssq[:])

            nc.gpsimd.dma_start(out=cc_in[:], in_=stats[:])
            nc.gpsimd.collective_compute(
                "AllGather",
                ALU.bypass,
                replica_groups=[list(range(NCORES))],
                ins=[cc_in.opt()],
                outs=[ag_out.ap().opt()],
            )
            nc.gpsimd.dma_start(
                out=gath[:],
                in_=ag_out.ap().rearrange("(g p) f -> p g f", p=128),
            )
            nc.vector.reduce_sum(
                statsR[:], gath[:].rearrange("p g f -> p f g"), axis=AX.X
            )
            # shsc = -2 * S (both halves)
            nc.vector.tensor_scalar_mul(shsc[:], statsR[:, 0:2 * C], -2.0)

            # ---- phase 3: per-row loss ----
            for u in range(T):
                lo, hi = u * 128, (u + 1) * 128
                pP = qp.tile([128, C], F32, tag="pP")
                # pP = -2*G + Ssq  (c-dependent part of A)
                nc.tensor.matmul(pP[:], lhsT=xt0[:, lo:hi], rhs=shsc[:, 0:C],
                                 start=True, stop=False)
                nc.tensor.matmul(pP[:], lhsT=xt1[:, lo:hi], rhs=shsc[:, C:2 * C],
                                 start=False, stop=False)
                nc.tensor.matmul(pP[:], lhsT=ones_row[:], rhs=statsR[0:1, 2 * C:3 * C],
                                 start=False, stop=True)

                # P_oth for all 7 columns; the own-class (leave-one-out)
                # value is an exact rescale: P_own = P_oth * M/(M-1), so the
                # select fuses into one multiply-add against the one-hot mask.
                p_oth = pc.tile([128, C], F32, tag="p_oth")
                nc.scalar.activation(p_oth[:], pP[:], AF.Identity,
                                     bias=b_oth[:, u:u + 1], scale=consts[:, 0:1])

                mask_u = ytile[:, u, :]
                # scr7raw = mask * p_oth  (only own column nonzero)
                scr7 = pc.tile([128, C], F32, tag="scr7")
                nc.vector.tensor_tensor(scr7[:], p_oth[:], mask_u, op=ALU.mult)
                # own value (pre-LOO): P_oth[own] = row-sum of scr7raw
                own_raw = pc.tile([128, 1], F32, tag="own_raw")
                nc.vector.reduce_sum(own_raw[:], scr7[:], axis=AX.X)
                # p_fin: own column scaled by M/(M-1) (the exact LOO value)
                sc2 = pc.tile([128, C], F32, tag="sc2")
                nc.vector.tensor_scalar_mul(sc2[:], scr7[:], 1.0 / (M - 1))
                p_fin = pc.tile([128, C], F32, tag="p_fin")
                nc.vector.tensor_add(p_fin[:], p_oth[:], sc2[:])

                nmx = pc.tile([128, 1], F32, tag="nmx")
                nc.vector.tensor_reduce(
                    out=nmx[:], in_=p_fin[:], axis=AX.X, op=ALU.max, negate=True
                )

                ex = pc.tile([128, C], F32, tag="ex")
                se = pc.tile([128, 1], F32, tag="se")
                nc.scalar.activation(ex[:], p_fin[:], AF.Exp,
                                     bias=nmx[:], scale=1.0, accum_out=se[:])
                lnse = pc.tile([128, 1], F32, tag="lnse")
                nc.scalar.activation(lnse[:], se[:], AF.Ln)

                # L = (lnse - nmx) - M/(M-1)*own_raw ; accL[:,u] = relu(L)
                s1 = pc.tile([128, 1], F32, tag="s1")
                nc.vector.tensor_sub(s1[:], lnse[:], nmx[:])
                ot = pc.tile([128, 1], F32, tag="ot")
                nc.vector.tensor_scalar_mul(ot[:], own_raw[:], -float(M) / (M - 1))
                l_u = pc.tile([128, 1], F32, tag="l_u")
                nc.vector.tensor_add(l_u[:], s1[:], ot[:])
                nc.vector.tensor_scalar_max(accL[:, u:u + 1], l_u[:], 0.0)

            # ---- reduce to scalar ----
            nc.vector.reduce_sum(accT[:], accL[:], axis=AX.X)
            nc.tensor.matmul(ploss[:], lhsT=accT[:], rhs=ones_col[:],
                             start=True, stop=True)
            nc.scalar.copy(out_s[:], ploss[:])
            nc.sync.dma_start(out=out_d[:, :], in_=out_s[:])

    nc.compile()
    return nc


_NC_CACHE = None


def _get_nc():
    global _NC_CACHE
    if _NC_CACHE is None:
        _NC_CACHE = build_program()
    return _NC_CACHE


def make_in_maps(embeddings, variance):
    X = np.ascontiguousarray(np.asarray(embeddings, dtype=np.float32))
    assert X.shape == (B, D), X.shape
    var = float(np.asarray(variance))

    labels = np.repeat(np.arange(C), M)  # reference ignores `target`
    Yfull = np.zeros((B, C), np.float32)
    Yfull[np.arange(B), labels] = 1.0

    consts = np.zeros((128, 4), np.float32)
    consts[:, 0] = -0.5 / (var * M)
    consts[:, 1] = -0.5 / (var * (M - 1))
    consts[:, 2] = -0.5 / var
    consts[:, 3] = -0.5 * M / (var * (M - 1))

    in_maps = []
    for k in range(NCORES):
        s = slice(k * R, (k + 1) * R)
        in_maps.append({
            "x": X[s],
            "xt": np.ascontiguousarray(X[s].T),
            "y": np.ascontiguousarray(Yfull[s]),
            "consts": consts,
        })
    return in_maps


def kernel(embeddings, target, variance):
    del target  # labels are balanced & class-sorted by construction (as in reference)
    nc = _get_nc()
    in_maps = make_in_maps(embeddings, variance)
    res = run_bass_kernel_spmd(nc, in_maps, list(range(NCORES)))
    total = 0.0
    for k in range(NCORES):
        total += float(res.results[k]["loss_part"][0, 0])
    return np.float32(total)
